# revision 1
# baseline (speedup 1.0000x reference)
"""Trainium2 Bass kernel for nn_MultiHeadAttention (B=4, T=2048, D=1024, H=16).

Sharding: 8 cores = 4 batches x 2 query-halves. Each core runs the full
attention for its 1024 queries against all 2048 keys (all 16 heads), so no
cross-core communication is needed; the host only concatenates the 8 output
slices. Odd cores receive a row-permuted x (their query half first) so the
same program runs on every core; attention is permutation-invariant over keys.

On-core layout: everything is computed feature-major ("transposed") so the
softmax feeds the PE directly:
  x^T (d on partitions)  ->  Q^T, K^T (feature-major), V (token-major)
  S^T[k, q] = K_h^T.T @ Q_h^T   (PE, contraction over head_dim=64,
                                 head pairs packed in partition halves)
  P^T = exp(S^T / 8)            (ACT, straight out of PSUM)
  O^T[d, q] += V_tile.T @ P^T   (PE, two heads packed via column groups)
  denom = ones.T @ sum_k P^T    (DVE accumulate + PE partition-sum)
  out[t, :] = (O^T / denom).T @ W_out + b_out
Matmuls run in float32r (single-pass PE mode, ~1.5e-4 rel err).
"""

import sys

sys.path.insert(0, "/opt/trn_rl_repo")

import numpy as np

B, T, D = 4, 2048, 1024
H, HD = 16, 64
NCORES = 8
TQ = T // 2  # queries per core
NP = 128
KT = T // NP  # 16 key tiles
DC = D // NP  # 8 d_model chunks
PAIRS = H // 2  # 8 head pairs; pair p owns features [128p, 128p+128)

_CACHE = {}


def _build():
    import concourse.bacc as bacc
    import concourse.tile as tile
    from concourse import masks, mybir

    F32 = mybir.dt.float32
    F32R = mybir.dt.float32r
    BF16 = mybir.dt.bfloat16
    AF = mybir.ActivationFunctionType

    nc = bacc.Bacc("TRN2", target_bir_lowering=False, debug=False,
                   num_devices=NCORES)
    x_io = nc.dram_tensor("x", [T, D], F32, kind="ExternalInput").ap()
    wqkv_io = nc.dram_tensor("wqkv", [D, 3 * D], F32, kind="ExternalInput").ap()
    bqkv_io = nc.dram_tensor("bqkv", [3 * D], F32, kind="ExternalInput").ap()
    wout_io = nc.dram_tensor("wout", [D, D], F32, kind="ExternalInput").ap()
    bout_io = nc.dram_tensor("bout", [D], F32, kind="ExternalInput").ap()
    out_io = nc.dram_tensor("out", [TQ, D], F32, kind="ExternalOutput").ap()

    qspill = nc.dram_tensor("qspill", [D, TQ], F32R).ap()  # Q^T feature-major
    kspill = nc.dram_tensor("kspill", [D, T], F32R).ap()   # K^T feature-major
    vspill = nc.dram_tensor("vspill", [T, D], BF16).ap()   # V token-major (bf16)

    bq_col = bqkv_io.rearrange("(n o) -> n o", o=1)  # [3072, 1]
    bq_row = bqkv_io.rearrange("(o n) -> o n", o=1)  # [1, 3072]
    bout_row = bout_io.rearrange("(o n) -> o n", o=1)  # [1, 1024]

    with tile.TileContext(nc) as tc:
        with (
            tc.tile_pool(name="const", bufs=1) as cpool,
            tc.tile_pool(name="otres", bufs=1) as ot_pool,
        ):
            ident = cpool.tile([NP, NP], F32, name="ident")
            masks.make_identity(nc, ident[:])
            ones_row = cpool.tile([1, NP], F32, name="ones_row")
            nc.vector.memset(ones_row[:], 1.0)
            ones_col = cpool.tile([NP, 1], BF16, name="ones_col")
            nc.vector.memset(ones_col[:], 1.0)
            # ind16[P][h, c] = 1.0 iff head h of pair P owns column c
            ones64 = cpool.tile([1, 64], F32, name="ones64")
            nc.vector.memset(ones64[:], 1.0)
            ind16 = []
            for p in range(PAIRS):
                t = cpool.tile([H, NP], F32, name=f"ind16_{p}")
                nc.vector.memset(t[:], 0.0)
                nc.sync.dma_start(t[2 * p:2 * p + 1, 0:64], ones64[:])
                nc.sync.dma_start(t[2 * p + 1:2 * p + 2, 64:NP], ones64[:])
                ind16.append(t)

            # b_v and b_out broadcast to [128, D] via K=1 ones matmul
            bv_row_sb = cpool.tile([1, D], F32, name="bv_row_sb")
            nc.sync.dma_start(bv_row_sb[:], bq_row[:, 2 * D:3 * D])
            bo_row_sb = cpool.tile([1, D], F32, name="bo_row_sb")
            nc.sync.dma_start(bo_row_sb[:], bout_row[:])
            bv_bc = cpool.tile([NP, D], F32, name="bv_bc")
            bo_bc = cpool.tile([NP, D], F32, name="bo_bc")
            with tc.tile_pool(name="bc_ps", bufs=2, space="PSUM") as bc_ps_pool:
                for dst, src in ((bv_bc, bv_row_sb), (bo_bc, bo_row_sb)):
                    for c in range(2):
                        ps = bc_ps_pool.tile([NP, 512], F32, name="bcps", tag="bcps")
                        nc.tensor.matmul(ps[:], ones_row[:], src[:, c * 512:(c + 1) * 512])
                        nc.vector.tensor_copy(dst[:, c * 512:(c + 1) * 512], ps[:])

            # ---- Stage A: x -> x^T (f32r), via PE transpose ----
            # xT_all[p, dc*T + t] = x[t, dc*128 + p]
            with (
                tc.tile_pool(name="xT", bufs=1) as xT_pool,
                nc.named_scope("xT"),
            ):
                xT = xT_pool.tile([NP, DC * T], F32R, name="xT")
                xT_v = xT.rearrange("p (dc t) -> p dc t", dc=DC)
                with (
                    tc.tile_pool(name="xload", bufs=3) as xl_pool,
                    tc.tile_pool(name="tr_ps", bufs=3, space="PSUM") as tr_pool,
                ):
                    for j in range(KT):  # 16 t-tiles
                        xt = xl_pool.tile([NP, D], F32, name="xt", tag="xt")
                        nc.sync.dma_start(xt[:], x_io[j * NP:(j + 1) * NP, :])
                        ps = tr_pool.tile([NP, D], F32, name="trps", tag="trps")
                        for dc in range(DC):
                            nc.tensor.transpose(
                                ps[:, dc * NP:(dc + 1) * NP],
                                xt[:, dc * NP:(dc + 1) * NP], ident[:])
                        nc.vector.tensor_copy(
                            xT_v[:, :, j * NP:(j + 1) * NP],
                            ps.rearrange("p (dc t) -> p dc t", dc=DC))

                # ---- Stage B: QKV projections ----
                xT_f = xT_v  # [128, dc, t]
                with (
                    nc.named_scope("qkv"),
                    tc.tile_pool(name="wload", bufs=3) as wl_pool,
                    tc.tile_pool(name="wr", bufs=10) as wr_pool,
                    tc.tile_pool(name="bias", bufs=2) as bias_pool,
                    tc.tile_pool(name="qkv_ps", bufs=6, space="PSUM") as qkv_ps_pool,
                    tc.tile_pool(name="qkv_sb", bufs=2) as qkv_sb_pool,
                ):
                    # Q^T and K^T (feature-major): lhsT = W tile, rhs = x^T
                    for is_k in (False, True):
                        ncols = T if is_k else TQ
                        f0 = D if is_k else 0
                        spill = kspill if is_k else qspill
                        nm = "k" if is_k else "q"
                        for p in range(PAIRS):
                            wts = []
                            for dc in range(DC):
                                wtmp = wl_pool.tile([NP, NP], F32, name=f"w{nm}l{p}_{dc}", tag="wl")
                                nc.sync.dma_start(
                                    wtmp[:],
                                    wqkv_io[dc * NP:(dc + 1) * NP,
                                            f0 + p * NP:f0 + (p + 1) * NP])
                                wr = wr_pool.tile([NP, NP], F32R, name=f"w{nm}r{p}_{dc}", tag="wr")
                                nc.vector.tensor_copy(wr[:], wtmp[:])
                                wts.append(wr)
                            bias = bias_pool.tile([NP, 1], F32, name=f"b{nm}{p}", tag="bias")
                            nc.sync.dma_start(
                                bias[:], bq_col[f0 + p * NP:f0 + (p + 1) * NP, :])
                            sb = qkv_sb_pool.tile([NP, ncols], F32R, name=f"{nm}sb{p}", tag=f"sb{nm}")
                            nch = ncols // 512
                            pss = [qkv_ps_pool.tile([NP, 512], F32, name=f"{nm}ps{p}_{c}", tag="qkvps")
                                   for c in range(nch)]
                            for dc in range(DC):  # one weight load, nch matmuls
                                for c in range(nch):
                                    nc.tensor.matmul(
                                        pss[c][:], wts[dc][:],
                                        xT_f[:, dc, c * 512:(c + 1) * 512],
                                        start=(dc == 0), stop=(dc == DC - 1))
                            for c in range(nch):
                                nc.vector.tensor_scalar_add(
                                    sb[:, c * 512:(c + 1) * 512], pss[c][:], bias[:])
                            nc.sync.dma_start(
                                spill[p * NP:(p + 1) * NP, :], sb[:])

                    # V (token-major): lhsT = x^T tile (one load serves both
                    # 512-wide f chunks), rhs = W_v columns
                    wvs = []
                    for dc in range(DC):
                        wtmp = wl_pool.tile([NP, D], F32, name=f"wvl{dc}", tag="wvl")
                        nc.sync.dma_start(
                            wtmp[:], wqkv_io[dc * NP:(dc + 1) * NP, 2 * D:3 * D])
                        wr = wr_pool.tile([NP, D], F32R, name=f"wvr{dc}", tag="wvr")
                        nc.vector.tensor_copy(wr[:], wtmp[:])
                        wvs.append(wr)
                    for ti in range(KT):
                        pss = [qkv_ps_pool.tile([NP, 512], F32, name=f"vps{ti}_{c}", tag="qkvps")
                               for c in range(2)]
                        for dc in range(DC):
                            for c in range(2):
                                nc.tensor.matmul(
                                    pss[c][:], xT_f[:, dc, ti * NP:(ti + 1) * NP],
                                    wvs[dc][:, c * 512:(c + 1) * 512],
                                    start=(dc == 0), stop=(dc == DC - 1))
                        for c in range(2):
                            sb = qkv_sb_pool.tile([NP, 512], BF16, name=f"vsb{ti}_{c}", tag="sbv")
                            nc.vector.tensor_add(
                                sb[:], pss[c][:], bv_bc[:, c * 512:(c + 1) * 512])
                            nc.sync.dma_start(
                                vspill[ti * NP:(ti + 1) * NP,
                                       c * 512:(c + 1) * 512], sb[:])

            # ---- Stage C: attention per head pair ----
            oT = [ot_pool.tile([NP, TQ], F32R, name=f"oT{p}") for p in range(PAIRS)]
            vsp_v = vspill.rearrange("(i tp) f -> tp i f", tp=NP)
            with (
                nc.named_scope("attn"),
                tc.tile_pool(name="qt", bufs=2) as qt_pool,
                tc.tile_pool(name="kt", bufs=2) as kt_pool,
                tc.tile_pool(name="vt", bufs=2) as vt_pool,
                tc.tile_pool(name="pt", bufs=4) as pt_pool,
                tc.tile_pool(name="acc", bufs=2) as acc_pool,
                tc.tile_pool(name="rcp", bufs=1) as rcp_pool,
                tc.tile_pool(name="s_ps", bufs=3, space="PSUM") as s_pool,
                tc.tile_pool(name="o_ps", bufs=1, space="PSUM") as o_pool,
            ):
                recip_in = rcp_pool.tile([H, TQ], F32, name="recip_in")
                recip_out = rcp_pool.tile([H, TQ], F32, name="recip_out")
                for p in range(PAIRS):
                    qt = qt_pool.tile([NP, TQ], F32R, name=f"qt{p}", tag="qt")
                    nc.sync.dma_start(qt[:], qspill[p * NP:(p + 1) * NP, :])
                    kt = kt_pool.tile([NP, T], F32R, name=f"kt{p}", tag="kt")
                    nc.sync.dma_start(kt[:], kspill[p * NP:(p + 1) * NP, :])
                    vt = vt_pool.tile([NP, KT * NP], BF16, name=f"vt{p}", tag="vt")
                    nc.sync.dma_start(
                        vt.rearrange("tp (i c) -> tp i c", i=KT),
                        vsp_v[:, :, p * NP:(p + 1) * NP])

                    ops = o_pool.tile([NP, TQ], F32, name=f"ops{p}", tag="ops")
                    accA = acc_pool.tile([NP, TQ], BF16, name=f"accA{p}", tag="accA")
                    accB = acc_pool.tile([NP, TQ], BF16, name=f"accB{p}", tag="accB")
                    for i in range(KT):
                        sA = s_pool.tile([NP, TQ], F32, name=f"sA{p}_{i}", tag="s")
                        sB = s_pool.tile([NP, TQ], F32, name=f"sB{p}_{i}", tag="s")
                        for c in range(2):
                            nc.tensor.matmul(
                                sA[:, c * 512:(c + 1) * 512],
                                kt[0:HD, i * NP:(i + 1) * NP],
                                qt[0:HD, c * 512:(c + 1) * 512])
                        for c in range(2):
                            nc.tensor.matmul(
                                sB[:, c * 512:(c + 1) * 512],
                                kt[HD:NP, i * NP:(i + 1) * NP],
                                qt[HD:NP, c * 512:(c + 1) * 512])
                        pA = pt_pool.tile([NP, TQ], BF16, name=f"pA{p}_{i}", tag="pt")
                        pB = pt_pool.tile([NP, TQ], BF16, name=f"pB{p}_{i}", tag="pt")
                        nc.scalar.activation(pA[:], sA[:], AF.Exp, scale=0.125)
                        nc.scalar.activation(pB[:], sB[:], AF.Exp, scale=0.125)
                        if i == 0:
                            nc.vector.tensor_copy(accA[:], pA[:])
                            nc.vector.tensor_copy(accB[:], pB[:])
                        else:
                            nc.vector.tensor_add(accA[:], accA[:], pA[:])
                            nc.vector.tensor_add(accB[:], accB[:], pB[:])
                        # col-packed heads share PSUM banks; the sim's
                        # bank-granular group check false-positives here
                        for c in range(2):
                            nc.tensor.matmul(
                                ops[0:HD, c * 512:(c + 1) * 512],
                                vt[:, i * NP:i * NP + HD],
                                pA[:, c * 512:(c + 1) * 512],
                                start=(i == 0), stop=(i == KT - 1),
                                skip_group_check=True)
                        for c in range(2):
                            nc.tensor.matmul(
                                ops[HD:NP, c * 512:(c + 1) * 512],
                                vt[:, i * NP + HD:(i + 1) * NP],
                                pB[:, c * 512:(c + 1) * 512],
                                start=(i == 0), stop=(i == KT - 1),
                                skip_group_check=True)

                    nc.vector.tensor_copy(oT[p][:], ops[:])
                    # denominators: partition-sum of acc via ones matmul,
                    # DMA the [1, TQ] rows straight out of PSUM; the
                    # reciprocal + normalize run once after all pairs.
                    for h, acc in ((0, accA), (1, accB)):
                        dn = o_pool.tile([NP, TQ], F32, name=f"dn{p}_{h}", tag="ops")
                        for c in range(2):
                            nc.tensor.matmul(
                                dn[0:1, c * 512:(c + 1) * 512], ones_col[:],
                                acc[:, c * 512:(c + 1) * 512])
                        dnr = rcp_pool.tile([1, TQ], F32, name=f"dnr{p}_{h}",
                                            tag="dnr", bufs=4)
                        nc.vector.tensor_copy(dnr[:], dn[0:1, :])
                        nc.sync.dma_start(recip_in[2 * p + h:2 * p + h + 1, :],
                                          dnr[:])

                # deferred normalization: one batched reciprocal, then
                # per-pair broadcast matmul + multiply
                nc.vector.reciprocal(recip_out[:], recip_in[:])
                for p in range(PAIRS):
                    rbc = s_pool.tile([NP, TQ], F32, name=f"rbc{p}", tag="s")
                    for c in range(2):
                        nc.tensor.matmul(
                            rbc[:, c * 512:(c + 1) * 512], ind16[p][:],
                            recip_out[:, c * 512:(c + 1) * 512])
                    nc.vector.tensor_mul(oT[p][:], oT[p].bitcast(F32)[:], rbc[:])

            # ---- Stage D: out projection ----
            with (
                nc.named_scope("outproj"),
                tc.tile_pool(name="wo", bufs=1) as wo_pool,
                tc.tile_pool(name="wol", bufs=2) as wol_pool,
                tc.tile_pool(name="f_ps", bufs=4, space="PSUM") as f_ps_pool,
                tc.tile_pool(name="f_sb", bufs=3) as f_sb_pool,
            ):
                wos = []
                for p in range(PAIRS):
                    wtmp = wol_pool.tile([NP, D], F32, name=f"wol{p}", tag="wol")
                    nc.sync.dma_start(wtmp[:], wout_io[p * NP:(p + 1) * NP, :])
                    wo = wo_pool.tile([NP, D], F32R, name=f"wo{p}")
                    nc.vector.tensor_copy(wo[:], wtmp[:])
                    wos.append(wo)
                for tj in range(TQ // NP):
                    fsb = f_sb_pool.tile([NP, D], F32, name=f"fsb{tj}", tag="fsb")
                    pss = [f_ps_pool.tile([NP, 512], F32, name=f"fps{tj}_{c}", tag="fps")
                           for c in range(2)]
                    for p in range(PAIRS):
                        for c in range(2):
                            nc.tensor.matmul(
                                pss[c][:], oT[p][:, tj * NP:(tj + 1) * NP],
                                wos[p][:, c * 512:(c + 1) * 512],
                                start=(p == 0), stop=(p == PAIRS - 1))
                    for c in range(2):
                        nc.vector.tensor_add(
                            fsb[:, c * 512:(c + 1) * 512], pss[c][:],
                            bo_bc[:, c * 512:(c + 1) * 512])
                    nc.sync.dma_start(out_io[tj * NP:(tj + 1) * NP, :], fsb[:])

    nc.compile()
    return nc


def get_nc():
    if "nc" not in _CACHE:
        _CACHE["nc"] = _build()
    return _CACHE["nc"]


def make_in_maps(x, W_qkv, b_qkv, W_out, b_out):
    x = np.ascontiguousarray(np.asarray(x, dtype=np.float32))
    W_qkv = np.ascontiguousarray(np.asarray(W_qkv, dtype=np.float32))
    b_qkv = np.ascontiguousarray(np.asarray(b_qkv, dtype=np.float32))
    W_out = np.ascontiguousarray(np.asarray(W_out, dtype=np.float32))
    b_out = np.ascontiguousarray(np.asarray(b_out, dtype=np.float32))
    in_maps = []
    for core in range(NCORES):
        b, half = divmod(core, 2)
        xb = x[b]
        if half == 1:  # put this core's query rows first; key order is free
            xb = np.concatenate([xb[TQ:], xb[:TQ]], axis=0)
        in_maps.append({
            "x": np.ascontiguousarray(xb),
            "wqkv": W_qkv, "bqkv": b_qkv, "wout": W_out, "bout": b_out,
        })
    return in_maps


def run(in_maps, trace=False):
    from concourse.bass_utils import run_bass_kernel_spmd
    nc = get_nc()
    return run_bass_kernel_spmd(nc, in_maps, list(range(NCORES)), trace=trace)


def kernel(x, W_qkv, b_qkv, W_out, b_out):
    res = run(make_in_maps(x, W_qkv, b_qkv, W_out, b_out))
    out = np.empty((B, T, D), dtype=np.float32)
    for core in range(NCORES):
        b, half = divmod(core, 2)
        out[b, half * TQ:(half + 1) * TQ] = res.results[core]["out"]
    return out



# revision 9
# speedup vs baseline: 1.4179x; 1.4179x over previous
"""Trainium2 Bass kernel for nn_MultiHeadAttention (B=4, T=2048, D=1024, H=16).

Sharding: 8 cores = 4 batches x 2 query-halves. Each core runs the full
attention for its 1024 queries against all 2048 keys (all 16 heads), so no
cross-core communication is needed; the host only concatenates the 8 output
slices. Odd cores receive a row-permuted x (their query half first) so the
same program runs on every core; attention is permutation-invariant over keys.

On-core layout: everything is computed feature-major ("transposed") so the
softmax feeds the PE directly:
  x^T (d on partitions)  ->  Q^T, K^T (feature-major bf16), V (token-major bf16)
  S^T[k, q] = ktH.T @ Q^T      (PE; ktH is the pair's K^T with the other
                                head's 64 feature rows zeroed, so every S
                                matmul contracts over the full 128 partitions
                                and keeps the PE array fully configured --
                                half-array matmuls pin the HAM clock gate at
                                1.2 GHz for the entire attention phase)
  P^T = exp(S^T / 8)           (ACT, straight out of PSUM)
  O^T[d, q] += vtH.T @ P^T     (PE; vtH is V for one head padded to 128
                                columns with a ones column, which accumulates
                                the softmax denominator for free on the PSUM
                                partition next to the head's 64 output dims)
  out[t, :] = (O^T / denom).T @ W_out + b_out
QKV/out projections run in float32r; attention runs in bf16.
"""

import sys

sys.path.insert(0, "/opt/trn_rl_repo")

import numpy as np

B, T, D = 4, 2048, 1024
H, HD = 16, 64
NCORES = 8
TQ = T // 2  # queries per core
NP = 128
KT = T // NP  # 16 key tiles
DC = D // NP  # 8 d_model chunks
PAIRS = H // 2  # 8 head pairs; pair p owns features [128p, 128p+128)

_CACHE = {}


def _build():
    import concourse.bacc as bacc
    import concourse.tile as tile
    from concourse import masks, mybir

    F32 = mybir.dt.float32
    F32R = mybir.dt.float32r
    BF16 = mybir.dt.bfloat16
    AF = mybir.ActivationFunctionType

    nc = bacc.Bacc("TRN2", target_bir_lowering=False, debug=False,
                   num_devices=NCORES)
    x_io = nc.dram_tensor("x", [T, D], F32, kind="ExternalInput").ap()
    wqkv_io = nc.dram_tensor("wqkv", [D, 3 * D], F32, kind="ExternalInput").ap()
    bqkv_io = nc.dram_tensor("bqkv", [3 * D], F32, kind="ExternalInput").ap()
    wout_io = nc.dram_tensor("wout", [D, D], F32, kind="ExternalInput").ap()
    bout_io = nc.dram_tensor("bout", [D], F32, kind="ExternalInput").ap()
    out_io = nc.dram_tensor("out", [TQ, D], F32, kind="ExternalOutput").ap()

    qspill = nc.dram_tensor("qspill", [D, TQ], BF16).ap()  # Q^T feature-major
    kspill = nc.dram_tensor("kspill", [D, T], BF16).ap()   # K^T feature-major
    vspill = nc.dram_tensor("vspill", [T, D], BF16).ap()   # V token-major

    bq_col = bqkv_io.rearrange("(n o) -> n o", o=1)  # [3072, 1]
    bq_row = bqkv_io.rearrange("(o n) -> o n", o=1)  # [1, 3072]
    bout_row = bout_io.rearrange("(o n) -> o n", o=1)  # [1, 1024]

    with tile.TileContext(nc) as tc:
        with (
            tc.tile_pool(name="const", bufs=1) as cpool,
            tc.tile_pool(name="otres", bufs=1) as ot_pool,
        ):
            ident = cpool.tile([NP, NP], F32, name="ident")
            masks.make_identity(nc, ident[:])
            ones_row = cpool.tile([1, NP], F32, name="ones_row")
            nc.vector.memset(ones_row[:], 1.0)
            # ind16[P][h, c] = 1.0 iff head h of pair P owns column c
            # (built in f32, shadow-copied to f32r: memset/DMA can't emit
            # f32r rounding, only DVE copies can)
            ones64 = cpool.tile([1, 64], F32, name="ones64")
            nc.vector.memset(ones64[:], 1.0)
            ind16 = []
            for p in range(PAIRS):
                tf = cpool.tile([H, NP], F32, name=f"ind16f_{p}")
                nc.vector.memset(tf[:], 0.0)
                nc.sync.dma_start(tf[2 * p:2 * p + 1, 0:64], ones64[:])
                nc.sync.dma_start(tf[2 * p + 1:2 * p + 2, 64:NP], ones64[:])
                t = cpool.tile([H, NP], F32R, name=f"ind16_{p}")
                nc.vector.tensor_copy(t[:], tf[:])
                ind16.append(t)

            # b_v and b_out broadcast to [128, D] via K=1 ones matmul
            bv_row_sb = cpool.tile([1, D], F32, name="bv_row_sb")
            nc.sync.dma_start(bv_row_sb[:], bq_row[:, 2 * D:3 * D])
            bo_row_sb = cpool.tile([1, D], F32, name="bo_row_sb")
            nc.sync.dma_start(bo_row_sb[:], bout_row[:])
            bv_bc = cpool.tile([NP, D], F32, name="bv_bc")
            bo_bc = cpool.tile([NP, D], F32, name="bo_bc")
            with tc.tile_pool(name="bc_ps", bufs=2, space="PSUM") as bc_ps_pool:
                for dst, src in ((bv_bc, bv_row_sb), (bo_bc, bo_row_sb)):
                    for c in range(2):
                        ps = bc_ps_pool.tile([NP, 512], F32, name="bcps", tag="bcps")
                        nc.tensor.matmul(ps[:], ones_row[:], src[:, c * 512:(c + 1) * 512])
                        nc.vector.tensor_copy(dst[:, c * 512:(c + 1) * 512], ps[:])

            # ---- Stage A: x -> x^T (f32r), via PE transpose ----
            # xT_all[p, dc*T + t] = x[t, dc*128 + p]
            with (
                tc.tile_pool(name="xT", bufs=1) as xT_pool,
                nc.named_scope("xT"),
            ):
                xT = xT_pool.tile([NP, DC * T], F32R, name="xT")
                xT_v = xT.rearrange("p (dc t) -> p dc t", dc=DC)
                with (
                    tc.tile_pool(name="xload", bufs=3) as xl_pool,
                    tc.tile_pool(name="tr_ps", bufs=3, space="PSUM") as tr_pool,
                ):
                    for j in range(KT):  # 16 t-tiles
                        xt = xl_pool.tile([NP, D], F32, name="xt", tag="xt")
                        nc.sync.dma_start(xt[:], x_io[j * NP:(j + 1) * NP, :])
                        ps = tr_pool.tile([NP, D], F32, name="trps", tag="trps")
                        for dc in range(DC):
                            nc.tensor.transpose(
                                ps[:, dc * NP:(dc + 1) * NP],
                                xt[:, dc * NP:(dc + 1) * NP], ident[:])
                        nc.vector.tensor_copy(
                            xT_v[:, :, j * NP:(j + 1) * NP],
                            ps.rearrange("p (dc t) -> p dc t", dc=DC))

                # ---- Stage B: QKV projections ----
                xT_f = xT_v  # [128, dc, t]
                with (
                    nc.named_scope("qkv"),
                    tc.tile_pool(name="wload", bufs=3) as wl_pool,
                    tc.tile_pool(name="wr", bufs=10) as wr_pool,
                    tc.tile_pool(name="bias", bufs=2) as bias_pool,
                    tc.tile_pool(name="qkv_ps", bufs=6, space="PSUM") as qkv_ps_pool,
                    tc.tile_pool(name="qkv_sb", bufs=2) as qkv_sb_pool,
                ):
                    # Q^T and K^T (feature-major): lhsT = W tile, rhs = x^T
                    for is_k in (False, True):
                        ncols = T if is_k else TQ
                        f0 = D if is_k else 0
                        spill = kspill if is_k else qspill
                        nm = "k" if is_k else "q"
                        for p in range(PAIRS):
                            wts = []
                            for dc in range(DC):
                                wtmp = wl_pool.tile([NP, NP], F32, name=f"w{nm}l{p}_{dc}", tag="wl")
                                nc.sync.dma_start(
                                    wtmp[:],
                                    wqkv_io[dc * NP:(dc + 1) * NP,
                                            f0 + p * NP:f0 + (p + 1) * NP])
                                wr = wr_pool.tile([NP, NP], F32R, name=f"w{nm}r{p}_{dc}", tag="wr")
                                nc.vector.tensor_copy(wr[:], wtmp[:])
                                wts.append(wr)
                            bias = bias_pool.tile([NP, 1], F32, name=f"b{nm}{p}", tag="bias")
                            nc.sync.dma_start(
                                bias[:], bq_col[f0 + p * NP:f0 + (p + 1) * NP, :])
                            sb = qkv_sb_pool.tile([NP, ncols], BF16, name=f"{nm}sb{p}", tag=f"sb{nm}")
                            nch = ncols // 512
                            pss = [qkv_ps_pool.tile([NP, 512], F32, name=f"{nm}ps{p}_{c}", tag="qkvps")
                                   for c in range(nch)]
                            for dc in range(DC):  # one weight load, nch matmuls
                                for c in range(nch):
                                    nc.tensor.matmul(
                                        pss[c][:], wts[dc][:],
                                        xT_f[:, dc, c * 512:(c + 1) * 512],
                                        start=(dc == 0), stop=(dc == DC - 1))
                            for c in range(nch):
                                nc.vector.tensor_scalar_add(
                                    sb[:, c * 512:(c + 1) * 512], pss[c][:], bias[:])
                            nc.sync.dma_start(
                                spill[p * NP:(p + 1) * NP, :], sb[:])

                    # V (token-major): lhsT = x^T tile (one load serves both
                    # 512-wide f chunks), rhs = W_v columns
                    wvs = []
                    for dc in range(DC):
                        wtmp = wl_pool.tile([NP, D], F32, name=f"wvl{dc}", tag="wvl")
                        nc.sync.dma_start(
                            wtmp[:], wqkv_io[dc * NP:(dc + 1) * NP, 2 * D:3 * D])
                        wr = wr_pool.tile([NP, D], F32R, name=f"wvr{dc}", tag="wvr")
                        nc.vector.tensor_copy(wr[:], wtmp[:])
                        wvs.append(wr)
                    for ti in range(KT):
                        pss = [qkv_ps_pool.tile([NP, 512], F32, name=f"vps{ti}_{c}", tag="qkvps")
                               for c in range(2)]
                        for dc in range(DC):
                            for c in range(2):
                                nc.tensor.matmul(
                                    pss[c][:], xT_f[:, dc, ti * NP:(ti + 1) * NP],
                                    wvs[dc][:, c * 512:(c + 1) * 512],
                                    start=(dc == 0), stop=(dc == DC - 1))
                        for c in range(2):
                            sb = qkv_sb_pool.tile([NP, 512], BF16, name=f"vsb{ti}_{c}", tag="sbv")
                            nc.vector.tensor_add(
                                sb[:], pss[c][:], bv_bc[:, c * 512:(c + 1) * 512])
                            nc.sync.dma_start(
                                vspill[ti * NP:(ti + 1) * NP,
                                       c * 512:(c + 1) * 512], sb[:])

            # ---- Stage C: attention per head pair ----
            # All matmuls contract over the full 128 partitions (zero-padded
            # K^T per head) and produce 128 output partitions (zero/ones
            # padded V per head) so the PE array stays fully configured.
            oT = [ot_pool.tile([NP, TQ], F32R, name=f"oT{p}") for p in range(PAIRS)]
            vsp_v = vspill.rearrange("(i tp) f -> tp i f", tp=NP)
            with (
                nc.named_scope("attn"),
                tc.tile_pool(name="qt", bufs=2) as qt_pool,
                tc.tile_pool(name="kt", bufs=4) as kt_pool,
                tc.tile_pool(name="vt", bufs=4) as vt_pool,
                tc.tile_pool(name="pt", bufs=4) as pt_pool,
                tc.tile_pool(name="rcp", bufs=1) as rcp_pool,
                tc.tile_pool(name="s_ps", bufs=2, space="PSUM") as s_pool,
                tc.tile_pool(name="o_ps", bufs=2, space="PSUM") as o_pool,
            ):
                recip_in = rcp_pool.tile([H, TQ], F32, name="recip_in")
                recip_f32 = rcp_pool.tile([H, TQ], F32, name="recip_f32")
                recip_out = rcp_pool.tile([H, TQ], F32R, name="recip_out")
                for p in range(PAIRS):
                    qt = qt_pool.tile([NP, TQ], BF16, name=f"qt{p}", tag="qt")
                    nc.sync.dma_start(qt[:], qspill[p * NP:(p + 1) * NP, :])
                    # K^T per head, other head's 64 feature rows zeroed
                    ktA = kt_pool.tile([NP, T], BF16, name=f"ktA{p}", tag="kt")
                    nc.vector.memset(ktA[HD:NP, :], 0.0)
                    nc.sync.dma_start(ktA[0:HD, :],
                                      kspill[p * NP:p * NP + HD, :])
                    ktB = kt_pool.tile([NP, T], BF16, name=f"ktB{p}", tag="kt")
                    nc.vector.memset(ktB[0:HD, :], 0.0)
                    nc.sync.dma_start(ktB[HD:NP, :],
                                      kspill[p * NP + HD:(p + 1) * NP, :])
                    # V per head padded to 128 stationary columns:
                    #   vtA cols [0:64) = V_A, col 64 = ones (denominator),
                    #   vtB col 0 = ones, cols [64:128) = V_B.
                    vtA = vt_pool.tile([NP, KT * NP], BF16, name=f"vtA{p}", tag="vt")
                    vtA_r = vtA.rearrange("tp (i c) -> tp i c", i=KT)
                    nc.vector.memset(vtA[:], 0.0)
                    nc.vector.memset(vtA_r[:, :, HD:HD + 1], 1.0)
                    nc.sync.dma_start(vtA_r[:, :, 0:HD],
                                      vsp_v[:, :, p * NP:p * NP + HD])
                    vtB = vt_pool.tile([NP, KT * NP], BF16, name=f"vtB{p}", tag="vt")
                    vtB_r = vtB.rearrange("tp (i c) -> tp i c", i=KT)
                    nc.vector.memset(vtB[:], 0.0)
                    nc.vector.memset(vtB_r[:, :, 0:1], 1.0)
                    nc.sync.dma_start(vtB_r[:, :, HD:NP],
                                      vsp_v[:, :, p * NP + HD:(p + 1) * NP])

                    opsA = o_pool.tile([NP, TQ], F32, name=f"opsA{p}", tag="ops")
                    opsB = o_pool.tile([NP, TQ], F32, name=f"opsB{p}", tag="ops")
                    for i in range(KT):
                        sA = s_pool.tile([NP, TQ], F32, name=f"sA{p}_{i}", tag="s")
                        sB = s_pool.tile([NP, TQ], F32, name=f"sB{p}_{i}", tag="s")
                        for c in range(2):
                            nc.tensor.matmul(
                                sA[:, c * 512:(c + 1) * 512],
                                ktA[:, i * NP:(i + 1) * NP],
                                qt[:, c * 512:(c + 1) * 512])
                        for c in range(2):
                            nc.tensor.matmul(
                                sB[:, c * 512:(c + 1) * 512],
                                ktB[:, i * NP:(i + 1) * NP],
                                qt[:, c * 512:(c + 1) * 512])
                        pA = pt_pool.tile([NP, TQ], BF16, name=f"pA{p}_{i}", tag="pt")
                        pB = pt_pool.tile([NP, TQ], BF16, name=f"pB{p}_{i}", tag="pt")
                        nc.scalar.activation(pA[:], sA[:], AF.Exp, scale=0.125)
                        nc.scalar.activation(pB[:], sB[:], AF.Exp, scale=0.125)
                        for c in range(2):
                            nc.tensor.matmul(
                                opsA[:, c * 512:(c + 1) * 512],
                                vtA[:, i * NP:(i + 1) * NP],
                                pA[:, c * 512:(c + 1) * 512],
                                start=(i == 0), stop=(i == KT - 1))
                        for c in range(2):
                            nc.tensor.matmul(
                                opsB[:, c * 512:(c + 1) * 512],
                                vtB[:, i * NP:(i + 1) * NP],
                                pB[:, c * 512:(c + 1) * 512],
                                start=(i == 0), stop=(i == KT - 1))

                    # O^T rows land partition-aligned with oT's pair packing
                    nc.vector.tensor_copy(oT[p][0:HD, :], opsA[0:HD, :])
                    nc.vector.tensor_copy(oT[p][HD:NP, :], opsB[HD:NP, :])
                    # denominators: partition 64 of opsA / partition 0 of opsB
                    dnr = rcp_pool.tile([NP, TQ], F32, name=f"dnr{p}",
                                        tag="dnr", bufs=2)
                    nc.vector.tensor_copy(dnr[HD:HD + 1, :], opsA[HD:HD + 1, :])
                    nc.vector.tensor_copy(dnr[0:1, :], opsB[0:1, :])
                    nc.sync.dma_start(recip_in[2 * p:2 * p + 1, :],
                                      dnr[HD:HD + 1, :])
                    nc.sync.dma_start(recip_in[2 * p + 1:2 * p + 2, :],
                                      dnr[0:1, :])

                # deferred normalization: one batched reciprocal, then
                # per-pair broadcast matmul + multiply
                nc.vector.reciprocal(recip_f32[:], recip_in[:])
                nc.vector.tensor_copy(recip_out[:], recip_f32[:])
                for p in range(PAIRS):
                    rbc = s_pool.tile([NP, TQ], F32, name=f"rbc{p}", tag="s")
                    for c in range(2):
                        nc.tensor.matmul(
                            rbc[:, c * 512:(c + 1) * 512],
                            ind16[p][:],
                            recip_out[:, c * 512:(c + 1) * 512])
                    nc.vector.tensor_mul(oT[p][:], oT[p].bitcast(F32)[:], rbc[:])

            # ---- Stage D: out projection ----
            with (
                nc.named_scope("outproj"),
                tc.tile_pool(name="wo", bufs=1) as wo_pool,
                tc.tile_pool(name="wol", bufs=2) as wol_pool,
                tc.tile_pool(name="f_ps", bufs=4, space="PSUM") as f_ps_pool,
                tc.tile_pool(name="f_sb", bufs=3) as f_sb_pool,
            ):
                wos = []
                for p in range(PAIRS):
                    wtmp = wol_pool.tile([NP, D], F32, name=f"wol{p}", tag="wol")
                    nc.sync.dma_start(wtmp[:], wout_io[p * NP:(p + 1) * NP, :])
                    wo = wo_pool.tile([NP, D], F32R, name=f"wo{p}")
                    nc.vector.tensor_copy(wo[:], wtmp[:])
                    wos.append(wo)
                for tj in range(TQ // NP):
                    fsb = f_sb_pool.tile([NP, D], F32, name=f"fsb{tj}", tag="fsb")
                    pss = [f_ps_pool.tile([NP, 512], F32, name=f"fps{tj}_{c}", tag="fps")
                           for c in range(2)]
                    for p in range(PAIRS):
                        for c in range(2):
                            nc.tensor.matmul(
                                pss[c][:], oT[p][:, tj * NP:(tj + 1) * NP],
                                wos[p][:, c * 512:(c + 1) * 512],
                                start=(p == 0), stop=(p == PAIRS - 1))
                    for c in range(2):
                        nc.vector.tensor_add(
                            fsb[:, c * 512:(c + 1) * 512], pss[c][:],
                            bo_bc[:, c * 512:(c + 1) * 512])
                    nc.sync.dma_start(out_io[tj * NP:(tj + 1) * NP, :], fsb[:])

    nc.compile()
    return nc


def get_nc():
    if "nc" not in _CACHE:
        _CACHE["nc"] = _build()
    return _CACHE["nc"]


def make_in_maps(x, W_qkv, b_qkv, W_out, b_out):
    x = np.ascontiguousarray(np.asarray(x, dtype=np.float32))
    W_qkv = np.ascontiguousarray(np.asarray(W_qkv, dtype=np.float32))
    b_qkv = np.ascontiguousarray(np.asarray(b_qkv, dtype=np.float32))
    W_out = np.ascontiguousarray(np.asarray(W_out, dtype=np.float32))
    b_out = np.ascontiguousarray(np.asarray(b_out, dtype=np.float32))
    in_maps = []
    for core in range(NCORES):
        b, half = divmod(core, 2)
        xb = x[b]
        if half == 1:  # put this core's query rows first; key order is free
            xb = np.concatenate([xb[TQ:], xb[:TQ]], axis=0)
        in_maps.append({
            "x": np.ascontiguousarray(xb),
            "wqkv": W_qkv, "bqkv": b_qkv, "wout": W_out, "bout": b_out,
        })
    return in_maps


def run(in_maps, trace=False):
    from concourse.bass_utils import run_bass_kernel_spmd
    nc = get_nc()
    return run_bass_kernel_spmd(nc, in_maps, list(range(NCORES)), trace=trace)


def kernel(x, W_qkv, b_qkv, W_out, b_out):
    res = run(make_in_maps(x, W_qkv, b_qkv, W_out, b_out))
    out = np.empty((B, T, D), dtype=np.float32)
    for core in range(NCORES):
        b, half = divmod(core, 2)
        out[b, half * TQ:(half + 1) * TQ] = res.results[core]["out"]
    return out
